# revision 24
# baseline (speedup 1.0000x reference)
"""Trainium2 Bass kernel for nn_BinaryLutLayer (embedding_lookup).

Per output row n (of 16384): addr = sum_b x[n,b] * 2^b (14 bits), then
y[n] = float32(luts_int[n, addr]).

Sharding: rows split across 8 cores (2048 rows each), no communication.

Per-core algorithm (all compute on device):
  1. addr = reduce_sum(x * 2^b) on the vector engine; exact in fp32.
  2. One int16 block index per row: idx = q*64 | (addr>>8), where q is
     the row's slot inside its 512-row chunk. All integer steps use
     bitwise ops (the DVE int path is an fp32 ALU, exact only to 2^24).
  3. 4x dma_gather (custom SWDGE gather): each fetches 512 rows' 256-byte
     LUT blocks (the block containing the addressed byte) from HBM -
     2048 descriptors total instead of reading the 32 MB LUT shard.
  4. Select the addressed halfword from each 256B block with an
     is_equal mask + multiply + reduce on the vector engine, then shift/
     mask out the byte and sign-extend - all exact bitwise arithmetic.

Host does layout-only work: row permutation so every DMA is contiguous,
LUT chunk slicing, and the int16 index-table wrap expected by the
gather firmware is produced on-device by one small SBUF->SBUF DMA.

Row mapping (core-local): slot (p, t) <-> row r = t*128 + p, chunk
c = t//4, j = t%4, in-chunk gather position q = j*128 + p.
"""

import numpy as np

NUM_BITS = 14
NUM_OUT = 16384
LUT_SIZE = 2**NUM_BITS
CORES = 8
NS = NUM_OUT // CORES  # rows per core = 2048
P = 128  # SBUF partitions
T = NS // P  # row-slots per partition = 16
NCHUNK = 4
CHUNK = NS // NCHUNK  # rows per dma_gather = 512
BLK = 256  # gather element size (bytes)
NBLK = CHUNK * (LUT_SIZE // BLK)  # blocks per LUT chunk = 32768

_CACHE: dict = {}


def _build_nc():
    import concourse.bacc as bacc
    import concourse.tile as tile
    from concourse import bass, mybir

    f32, i32, i16, i8, u16 = (
        mybir.dt.float32,
        mybir.dt.int32,
        mybir.dt.int16,
        mybir.dt.int8,
        mybir.dt.uint16,
    )
    Alu = mybir.AluOpType

    nc = bacc.Bacc(
        "TRN2",
        target_bir_lowering=False,
        debug=False,
        dynamic_dma_scratch_size=65536,
    )

    # x rows pre-permuted on host: DRAM row p*T+t holds logical row t*128+p,
    # so the load below is fully contiguous per partition.
    x_t = nc.dram_tensor("x_shard", [NS, NUM_BITS], f32, kind="ExternalInput")
    # second host permutation of x: DRAM row (t*8+u)*16 + qh holds logical
    # row t*128 + u*16 + qh. Addresses computed in this layout sit at
    # [partition t*8+u, col qh]; one PE transpose then lands them at
    # [partition qh, col t*8+u] - exactly the int16 index-table wrap the
    # gather firmware wants (partition q%16, col q//16 with q = j*128+p).
    x2_t = nc.dram_tensor("x2_shard", [NS, NUM_BITS], f32, kind="ExternalInput")
    lut_t = [
        nc.dram_tensor(f"lut{c}", [NBLK, BLK], i8, kind="ExternalInput")
        for c in range(NCHUNK)
    ]
    # consts: 0:7 = 2^(b-1) b=1..7, 7:13 = 2^(b-8) b=8..13, 13:141 = iota,
    # 141:269 = identity, 269:397 = qwrep, 397:525 = E (rows 0:16)
    co_t = nc.dram_tensor("consts", [P, 525], f32, kind="ExternalInput")
    y_t = nc.dram_tensor("y_shard", [NS, 1], f32, kind="ExternalOutput")

    with tile.TileContext(nc) as tc:
        with (
            tc.tile_pool(name="p", bufs=1) as pool,
            tc.tile_pool(name="ps", bufs=1, space="PSUM") as psum,
        ):
            x_sb = pool.tile([P, T * NUM_BITS], f32)
            x2_sb = pool.tile([P, T * NUM_BITS], f32)
            co_sb = pool.tile([P, 525], f32)
            prodh = pool.tile([P, T * 6], f32)
            prodk = pool.tile([P, T * 7], f32)
            hi2_f = pool.tile([P, T], f32)
            hiT_ps = psum.tile([16, P], f32)
            hiT_sb = pool.tile([16, P], f32)
            rep_ps = psum.tile([P, P], f32)
            idxw = pool.tile([P, P], i16)
            blocks = pool.tile([P, T * BLK], i8)
            k16_f = pool.tile([P, T], f32)
            par8 = pool.tile([P, T], i32)
            mask = pool.tile([P, T * (BLK // 2)], f32)
            msel = pool.tile([P, T * (BLK // 2)], f32)
            y16f = pool.tile([P, T], f32)
            h_i = pool.tile([P, T], i32)
            sh_i = pool.tile([P, T], i32)
            u8_i = pool.tile([P, T], i32)
            y_f = pool.tile([P, T], f32)

            # critical path first: x2 + consts feed the index pipeline;
            # three parallel HWDGE queues (sync / scalar / vector)
            nc.sync.dma_start(x2_sb[:], x2_t[:].rearrange("(p t) b -> p (t b)", p=P))
            nc.scalar.dma_start(co_sb[:], co_t[:])
            nc.gpsimd.dma_start(x_sb[:], x_t[:].rearrange("(p t) b -> p (t b)", p=P))

            w17 = co_sb[:, 0:7]  # 2^(b-1), b=1..7
            wh = co_sb[:, 7:13]  # 2^(b-8), b=8..13
            iota = co_sb[:, 13:141]  # [P,128] f32, value k
            ident = co_sb[:, 141:269]
            qw16 = co_sb[0:16, 269:397]  # q*64 for the 16 wrap partitions
            etile = co_sb[0:16, 397:525]  # E[k, m] = (m%16 == k)

            # hi = addr>>8 = sum_{b>=8} x_b 2^(b-8), computed directly
            x2h = x2_sb[:].rearrange("p (t b) -> p t b", b=NUM_BITS)[:, :, 8:14]
            wh3 = wh.rearrange("p b -> p () b").to_broadcast([P, T, 6])
            ph3 = prodh[:].rearrange("p (t b) -> p t b", b=6)
            nc.vector.tensor_tensor(out=ph3, in0=x2h, in1=wh3, op=Alu.mult)
            nc.vector.reduce_sum(out=hi2_f[:], in_=ph3, axis=mybir.AxisListType.X)

            # [pi=t*8+u, qh] -> [qh, pi]: lands hi at its wrap position
            nc.tensor.transpose(out=hiT_ps[:], in_=hi2_f[:], identity=ident)
            # full index value q*64 + hi, exact in fp32 (< 2^15)
            nc.vector.tensor_tensor(
                out=hiT_sb[:], in0=hiT_ps[:], in1=qw16, op=Alu.add
            )
            # replicate to all 8 gpsimd cores via matmul with a tiled identity
            nc.tensor.matmul(rep_ps[:], lhsT=etile, rhs=hiT_sb[:])
            nc.vector.tensor_copy(out=idxw[:], in_=rep_ps[:])

            # gathers: the critical resource, issue as soon as idxw lands
            blocks4 = blocks[:].rearrange("p (c j k) -> p c j k", c=NCHUNK, k=BLK)
            for c in range(NCHUNK):
                nc.gpsimd.dma_gather(
                    out_ap=blocks4[:, c],
                    in_ap=lut_t[c][:],
                    idxs_ap=idxw[:, c * 32 : (c + 1) * 32],
                    num_idxs=CHUNK,
                    num_idxs_reg=CHUNK,
                    elem_size=BLK,
                )

            # select path (off the gather critical path):
            # k16 = (addr>>1)&127 = sum_{b=1..7} x_b 2^(b-1); par8 = 8*x_0
            x3 = x_sb[:].rearrange("p (t b) -> p t b", b=NUM_BITS)
            w173 = w17.rearrange("p b -> p () b").to_broadcast([P, T, 7])
            pk3 = prodk[:].rearrange("p (t b) -> p t b", b=7)
            nc.vector.tensor_tensor(out=pk3, in0=x3[:, :, 1:8], in1=w173, op=Alu.mult)
            nc.vector.reduce_sum(out=k16_f[:], in_=pk3, axis=mybir.AxisListType.X)
            nc.vector.tensor_scalar(
                out=par8[:],
                in0=x3[:, :, 0:1].rearrange("p t one -> p (t one)"),
                scalar1=8.0, scalar2=None, op0=Alu.mult,
            )

            blocks_u16 = blocks[:].bitcast(u16).rearrange(
                "p (c j k) -> p c j k", c=NCHUNK, k=BLK // 2
            )
            mask4 = mask[:].rearrange("p (c j k) -> p c j k", c=NCHUNK, k=BLK // 2)
            msel4 = msel[:].rearrange("p (c j k) -> p c j k", c=NCHUNK, k=BLK // 2)
            iota_b = iota.rearrange("p k -> p () k").to_broadcast([P, NCHUNK, BLK // 2])
            k16_4 = k16_f[:].rearrange("p (c j) -> p c j", c=NCHUNK)

            for c in range(NCHUNK):
                kb = (
                    k16_4[:, c]
                    .rearrange("p j -> p j ()")
                    .to_broadcast([P, NCHUNK, BLK // 2])
                )
                nc.vector.tensor_tensor(
                    out=mask4[:, c], in0=iota_b, in1=kb, op=Alu.is_equal
                )
                nc.vector.tensor_tensor(
                    out=msel4[:, c], in0=mask4[:, c], in1=blocks_u16[:, c],
                    op=Alu.mult,
                )
                nc.vector.reduce_sum(
                    out=y16f[:, c * NCHUNK : (c + 1) * NCHUNK],
                    in_=msel4[:, c],
                    axis=mybir.AxisListType.X,
                )

            # byte extract + sign-extend, all exact bitwise ops
            nc.vector.tensor_copy(out=h_i[:], in_=y16f[:])
            nc.vector.tensor_tensor(
                out=sh_i[:], in0=h_i[:], in1=par8[:], op=Alu.logical_shift_right
            )
            nc.vector.tensor_scalar(
                out=u8_i[:], in0=sh_i[:], scalar1=255, scalar2=128,
                op0=Alu.bitwise_and, op1=Alu.bitwise_xor,
            )
            nc.vector.tensor_scalar(
                out=y_f[:], in0=u8_i[:], scalar1=128, scalar2=None,
                op0=Alu.subtract,
            )

            # store contiguous; host un-permutes (DRAM row p*T+t = slot (p,t))
            nc.sync.dma_start(y_t[:].rearrange("(p t) one -> p (t one)", p=P), y_f[:])

    nc.compile()
    return nc


def _get_nc():
    if "nc" not in _CACHE:
        _CACHE["nc"] = _build_nc()
    return _CACHE["nc"]


def _consts() -> np.ndarray:
    co = np.zeros((P, 525), dtype=np.float32)
    co[:, 0:7] = 2.0 ** np.arange(0, 7, dtype=np.float32)  # 2^(b-1), b=1..7
    co[:, 7:13] = 2.0 ** np.arange(0, 6, dtype=np.float32)  # 2^(b-8), b=8..13
    co[:, 13:141] = np.arange(P, dtype=np.float32)[None, :]
    co[:, 141:269] = np.eye(P, dtype=np.float32)
    # qwrep[16g+qh, pi] = (j*128 + u*16 + qh) * 64 with pi = c*32 + j*8 + u
    pi = np.arange(P)
    j, u = (pi % 32) // 8, pi % 8
    qh16 = np.arange(P) % 16
    co[:, 269:397] = ((j * P + u * 16)[None, :] + qh16[:, None]).astype(
        np.float32
    ) * 64.0
    # E[k, m] = (m % 16 == k)
    co[:16, 397:525] = (
        (np.arange(P)[None, :] % 16) == np.arange(16)[:, None]
    ).astype(np.float32)
    return co


def _make_in_maps(x, luts_int):
    co = _consts()
    x = np.asarray(x, dtype=np.float32).reshape(NUM_OUT, NUM_BITS)
    luts_int = np.asarray(luts_int, dtype=np.int8)
    in_maps = []
    for core in range(CORES):
        base = core * NS
        # permute rows: DRAM slot p*T+t <- logical row t*128+p
        xs = np.ascontiguousarray(
            x[base : base + NS].reshape(T, P, NUM_BITS).transpose(1, 0, 2)
        ).reshape(NS, NUM_BITS)
        # x2's layout (t*8+u)*16+qh == logical row order: no permutation
        x2 = np.ascontiguousarray(x[base : base + NS])
        m = {"x_shard": xs, "x2_shard": x2, "consts": co}
        for c in range(NCHUNK):
            m[f"lut{c}"] = luts_int[
                base + c * CHUNK : base + (c + 1) * CHUNK
            ].reshape(NBLK, BLK)
        in_maps.append(m)
    return in_maps


def kernel(x, luts_float, luts_int, _run_kwargs=None):
    from concourse.bass_utils import run_bass_kernel_spmd

    nc = _get_nc()
    in_maps = _make_in_maps(x, luts_int)
    res = run_bass_kernel_spmd(nc, in_maps, list(range(CORES)), **(_run_kwargs or {}))
    _CACHE["last_result"] = res
    out = np.empty((NUM_OUT, 1), dtype=np.float32)
    for core in range(CORES):
        ys = res.results[core]["y_shard"].reshape(P, T)  # [p, t]
        # logical row t*128+p = ys[p, t]
        out[core * NS : (core + 1) * NS, 0] = ys.T.reshape(NS)
    return out
